# revision 7
# baseline (speedup 1.0000x reference)
"""Trainium2 Bass kernel for nn_BasicBlockA — fp8 DoubleRow design.

Math (see reference):
  w1 = (weight1*mask0 + softplus(center1)*mask1) * mask      [16,3,3,3,3]
  h  = elu(conv2d(x, w1.reshape(48,3,3,3), pad=1) + bias1)   [B,48,H,W]
  h2 = grouped_conv(h, w2.reshape(48,3,3,3), groups=16)      [B,48,H,W]
  out = h2.reshape(B,16,3,H,W).mean(1) + res*(res>0)*x

Only 5 taps (0,0),(0,1),(0,2),(1,0),(1,1) survive the causal mask, in
BOTH convs.  Pure data parallel: 8 images per core on 8 cores.  Each
image is split into two 64-row halves (A = rows 0-63, B = 64-127)
packed as one "pixel pair" per PE column:

  stage 1 (bf16): one matmul per 4-row block (16/img), K=32 (15
    pre-shifted tap-planes + ones/bias row, per half), M=96 (48
    h-channels x 2 halves), out free 512.  The bias row carries
    bias1+1 so PSUM holds ps' = ps + 1.
  ELU' (2 engine stages, per 2-block chunk): ACT computes
    e = Exp(ps' - 1) (bias AP); DVE's fused scalar_tensor_tensor
    h' = max(min(e, 1), ps') = elu(ps)+1 writes fp8 h1b directly.
    Pad cells hold 1.0 (== h'=1, h=0) so stage 2 needs no edge cases;
    the exact -sum(w) correction for the +1 shift rides the outcopy's
    bias AP, and the host adds res*x.
  stage 2 (fp8e4 DoubleRow, 0.5 cyc/row): per 2-row pair g (32/img),
    3 matmuls of [K=96, 2 k-tiles, N=258].  N is a flat 258-window
    covering both pair rows (cols 128/129 dead, dropped by the host).
    The second k-tile reads a SHADOW copy of h1b (shifted one column,
    built by cheap gpsimd-queue DMAs per chunk), making every tile
    delta a 16B-aligned even stride as the DoubleRow ifmap streamer
    requires (s3_lw_dual_fp8_restrictions):
      pass0 main(2g+0,c0)  -> taps (0,0)+(0,1)        delta HB
      pass1 main(2g+1,c0)  -> taps (1,0)+(1,1)        delta HB
      pass2 main(2g+0,c2)  -> (0,2)+(1,1)-residual    delta HB+128
    (the residual tile compensates w2's fp8 quantization).  M=48 packs
    8 rotation sub-slots x (3 outs x 2 halves) at dst partition 0 (a
    hardware requirement); sub-slot rotation accumulates 8 pairs per
    [48, 258] PSUM region, zero-padded stationary columns make the
    rotation additive.  The B-half top halo row (and its shadow) is a
    partition-shifting SBUF->SBUF DMA.

PSUM: ps1 [96,6,512] (6 banks = 3 eltwise chunk groups), ps2
[48,2,512] (2 banks, ping-pong regions).  h1b [96, 2 img-buf,
2 copies, 8464] fp8 — copy pitch 8464 is the 16B-aligned tile delta.
Stage-1 runs blocks [12..15, 0..11] so the halo lands in chunk 1, off
stage-2's critical path; per-pair dvh/shd waits keep the PE from
over-waiting at image boundaries.
"""

import numpy as np

PERCORE = 8
N_CORES = 8
C, L, KK = 3, 16, 3
H = W = 128
CO1 = L * C  # 48
HALF = 64
NBLK1 = 16    # stage-1 4-row blocks per image
NPAIR = 32    # stage-2 2-row pairs per image
NCHUNK = 8    # eltwise chunks per image (2 blocks each)
TAPS = [(0, 0), (0, 1), (0, 2), (1, 0), (1, 1)]
# stage-2 pass tiles: (tap0, tap1); pass2 tile1 re-reads (1,1) for the
# fp8 weight-residual compensation.
P_TILES = [((0, 0), (0, 1)), ((1, 0), (1, 1)), ((0, 2), (1, 1))]

_CACHE = {}


def _softplus(x):
    return np.logaddexp(0.0, x)


def _make_masks(Cc, Kk):
    mid = Kk // 2
    mask0 = np.ones((Cc, Cc, Kk, Kk), np.float32)
    mask1 = np.zeros((Cc, Cc, Kk, Kk), np.float32)
    mask = np.ones((Cc, Cc, Kk, Kk), np.float32)
    for i in range(Cc):
        mask0[i, i, mid, mid] = 0.0
        mask1[i, i, mid, mid] = 1.0
        mask[i, :, mid + 1:, :] = 0.0
        mask[i, :i + 1, mid, mid + 1:] = 0.0
        mask[i, i + 1:, mid, mid:] = 0.0
    return mask0, mask1, mask


def _build_nc():
    import concourse.bass as bass
    import concourse.mybir as mybir

    f32 = mybir.dt.float32
    bf16 = mybir.dt.bfloat16
    fp8 = mybir.dt.float8e4
    AF = mybir.ActivationFunctionType
    ALU = mybir.AluOpType
    PM = mybir.MatmulPerfMode

    nc = bass.Bass()
    xs_t = nc.declare_dram_parameter("xs", [PERCORE, 32, HALF, W], bf16, False)
    w1_t = nc.declare_dram_parameter("w1", [32, 96], bf16, False)
    w2_t = nc.declare_dram_parameter("w2", [96, 8, 3, 2, 48], fp8, False)
    cb_t = nc.declare_dram_parameter("cb", [48, 1], f32, False)
    out_t = nc.declare_dram_parameter("out", [PERCORE, 48, 4, 258], f32, True)

    HROW = HALF + 1          # 65 rows per half-array (top halo/pad + 64)
    HCOL = W + 2             # 130
    HB = 8464                # copy pitch (>= 65*130 = 8450, multiple of 16)

    from contextlib import ExitStack
    with ExitStack() as ctx:
        xs_sb = ctx.enter_context(nc.sbuf_tensor("xs_sb", [32, 2, HALF, W], bf16))
        w1sb = ctx.enter_context(nc.sbuf_tensor("w1sb", [32, 96], bf16))
        w2sb = ctx.enter_context(nc.sbuf_tensor("w2sb", [96, 8, 3, 2, 48], fp8))
        cbsb = ctx.enter_context(nc.sbuf_tensor("cbsb", [48, 1], f32))
        # h' storage: [buf, copy, flat]; copy 0 = main, copy 1 = shadow
        # (main shifted one column).  Copy pitch HB is 16B aligned so the
        # DoubleRow tile deltas (HB, HB+128) satisfy the ifmap streamer.
        h1b = ctx.enter_context(nc.sbuf_tensor("h1b", [96, 2, 2, HB], fp8))
        e_sb = ctx.enter_context(nc.sbuf_tensor("e_sb", [96, 6, 512], bf16))
        out_sb = ctx.enter_context(nc.sbuf_tensor("out_sb", [48, 4, 258], f32))
        negone = ctx.enter_context(nc.sbuf_tensor("negone", [96, 1], f32))
        tdum = ctx.enter_context(nc.sbuf_tensor("tdum", [1, 2], bf16))
        ps1 = ctx.enter_context(nc.psum_tensor("ps1", [96, 6, 512], f32))
        ps2 = ctx.enter_context(nc.psum_tensor("ps2", [48, 2, 512], f32))
        wdma = ctx.enter_context(nc.semaphore("wdma"))
        wdma2 = ctx.enter_context(nc.semaphore("wdma2"))
        wdma3 = ctx.enter_context(nc.semaphore("wdma3"))
        xdma = ctx.enter_context(nc.semaphore("xdma"))
        odma = ctx.enter_context(nc.semaphore("odma"))
        mset = ctx.enter_context(nc.semaphore("mset"))
        s1pe = ctx.enter_context(nc.semaphore("s1pe"))
        s2pe = ctx.enter_context(nc.semaphore("s2pe"))
        acte = ctx.enter_context(nc.semaphore("acte"))
        dvh = ctx.enter_context(nc.semaphore("dvh"))
        halo = ctx.enter_context(nc.semaphore("halo"))
        ocp = ctx.enter_context(nc.semaphore("ocp"))
        shd = ctx.enter_context(nc.semaphore("shd"))
        block = ctx.enter_context(nc.Block())

        PSTRIDE = 2 * 2 * HB             # per-partition h1b elements
        # s1 processes the tail blocks first so the halo row (block 15)
        # lands in eltwise chunk 1, off stage-2's critical path.
        POS2BLK = [12, 13, 14, 15] + list(range(12))
        BLK2CHUNK = {b: p // 2 for p, b in enumerate(POS2BLK)}

        def pair_chunk(g):
            """Last eltwise chunk pair g depends on (within its image)."""
            blocks = {min(2 * g + 2, 63) // 4}
            if 2 * g - 1 >= 0:
                blocks.add((2 * g - 1) // 4)
            return max(BLK2CHUNK[b] for b in blocks)

        def cap(base, dims):
            ap = base.copy()
            ap.ap = type(ap.ap)(dims)
            return ap

        def s2mov(buf, g, p):
            """Moving AP for stage-2 pair g, pass p: [96, 2(tiles), 258].
            N is a flat 258-window spanning both pair rows; cols 128/129 are
            dead.  tile1 = the shadow copy (h shifted one column), so the
            tile delta is a 16B-aligned copy-pitch distance:
              p0: main(2g+0, 0) -> taps (0,0),(0,1)          delta HB
              p1: main(2g+1, 0) -> taps (1,0),(1,1)          delta HB
              p2: main(2g+0, 2) -> (0,2) + shadow(2g+1, 0) ->
                  (1,1)-residual                             delta HB+128"""
            dy0, dx0 = [(0, 0), (1, 0), (0, 2)][p]
            delta = HB if p < 2 else HB + HCOL - 2
            X = (2 * g + dy0) * HCOL + dx0
            return cap(h1b[0:96, buf, 0, X:X + 258],
                       [[PSTRIDE, 96], [delta, 2], [1, 258]])

        @block.sync
        def _(sync):
            # image 0 lands in four quarters, blocks-12-15 rows first,
            # so the PE can start early
            for n_, q in enumerate((3, 0, 1, 2)):
                sync.wait_ge(xdma, 16 * n_)
                sync.dma_start(
                    out=xs_sb[0:32, 0, 16 * q:16 * q + 16, :].opt(),
                    in_=xs_t[0, :, 16 * q:16 * q + 16, :].opt()
                    ).then_inc(xdma, 16)
            sync.wait_ge(xdma, 64)
            sync.dma_start(out=xs_sb[0:32, 1].opt(),
                           in_=xs_t[1].opt()).then_inc(xdma, 16)
            sync.wait_ge(mset, 6)
            for i in range(PERCORE):
                # halo: B-half top row <- A-half last row (partition shift);
                # block 15 lands in chunk 1 thanks to POS2BLK.
                sync.wait_ge(dvh, NCHUNK * i + 2)
                sync.wait_ge(halo, 32 * i)
                sync.dma_start(
                    out=h1b[48:96, i % 2, 0, 0:HCOL],
                    in_=h1b[0:48, i % 2, 0, HALF * HCOL:HALF * HCOL + HCOL]
                    ).then_inc(halo, 16)
                sync.wait_ge(halo, 32 * i + 16)
                sync.dma_start(out=h1b[48:96, i % 2, 1, 0:HCOL - 1],
                               in_=h1b[48:96, i % 2, 0, 1:HCOL]
                               ).then_inc(halo, 16)
                if i >= 1:
                    # out DMAs for image i-1 (copies fire mid/end of s2(i-1))
                    for s in range(2):
                        sync.wait_ge(ocp, 4 * (i - 1) + 2 * (s + 1))
                        sync.wait_ge(odma, 16 * (2 * (i - 1) + s))
                        sync.dma_start(
                            out=out_t[i - 1, :, 2 * s:2 * s + 2, :].opt(),
                            in_=out_sb[0:48, 2 * s:2 * s + 2, :].opt()
                            ).then_inc(odma, 16)
                if i + 2 < PERCORE:
                    sync.wait_ge(s1pe, NBLK1 * (i + 1))
                    sync.wait_ge(xdma, 48 + 16 * (i + 2))
                    sync.dma_start(out=xs_sb[0:32, (i + 2) % 2].opt(),
                                   in_=xs_t[i + 2].opt()).then_inc(xdma, 16)
            for s in range(2):
                sync.wait_ge(ocp, 4 * (PERCORE - 1) + 2 * (s + 1))
                sync.wait_ge(odma, 16 * (2 * (PERCORE - 1) + s))
                sync.dma_start(
                    out=out_t[PERCORE - 1, :, 2 * s:2 * s + 2, :].opt(),
                    in_=out_sb[0:48, 2 * s:2 * s + 2, :].opt()
                    ).then_inc(odma, 16)

        @block.gpsimd
        def _(gp):
            # negone first (gates ACT's exps), then the weight DMAs
            # (w1 gates the PE start; w2/cb only matter at stage 2),
            # then the h1b pad memsets (gate stage-2 reads + shadows)
            BUFD = [2 * HB, 2]
            nc.gpsimd.memset(negone[0:96], -1.0).then_inc(mset, 1)
            gp.dma_start(out=w1sb[0:32].opt(),
                         in_=w1_t[:].opt()).then_inc(wdma, 16)
            gp.dma_start(out=w2sb[0:96].opt(),
                         in_=w2_t[:].opt()).then_inc(wdma2, 16)
            gp.dma_start(out=cbsb[0:48].opt(),
                         in_=cb_t[:].opt()).then_inc(wdma3, 16)
            gp.wait_ge(mset, 1)
            nc.gpsimd.memset(cap(h1b[0:48, 0, 0, 0:HCOL],
                                 [[PSTRIDE, 48], BUFD, [1, HCOL]]),
                             1.0).then_inc(mset, 1)
            gp.wait_ge(mset, 2)
            nc.gpsimd.memset(cap(h1b[0:96, 0, 0, 0:1],
                                 [[PSTRIDE, 96], BUFD, [HCOL, HROW], [1, 1]]),
                             1.0).then_inc(mset, 1)
            gp.wait_ge(mset, 3)
            nc.gpsimd.memset(cap(h1b[0:96, 0, 0, HCOL - 1:HCOL],
                                 [[PSTRIDE, 96], BUFD, [HCOL, HROW], [1, 1]]),
                             1.0).then_inc(mset, 1)
            gp.wait_ge(mset, 4)
            nc.gpsimd.memset(cap(h1b[0:96, 0, 1, HCOL - 1:HCOL],
                                 [[PSTRIDE, 96], BUFD, [HCOL, HROW], [1, 1]]),
                             1.0).then_inc(mset, 1)
            gp.wait_ge(mset, 5)
            nc.gpsimd.memset(cap(h1b[0:48, 0, 1, 0:HCOL - 1],
                                 [[PSTRIDE, 48], BUFD, [1, HCOL - 1]]),
                             1.0).then_inc(mset, 1)
            # shadow builder: per eltwise chunk, copy the 8 fresh h rows
            # shifted one column into h1s
            for i in range(PERCORE):
                for k in range(NCHUNK):
                    gc = NCHUNK * i + k
                    gp.wait_ge(dvh, gc + 1)
                    gp.wait_ge(shd, 16 * gc)
                    b0 = POS2BLK[2 * k]
                    X = (4 * b0 + 1) * HCOL
                    gp.dma_start(
                        out=cap(h1b[0:96, i % 2, 1, X:X + HCOL - 1],
                                [[PSTRIDE, 96], [HCOL, 8], [1, HCOL - 1]]),
                        in_=cap(h1b[0:96, i % 2, 0, X + 1:X + HCOL],
                                [[PSTRIDE, 96], [HCOL, 8], [1, HCOL - 1]])
                        ).then_inc(shd, 16)

        @block.tensor
        def _(tensor):
            tensor.wait_ge(wdma, 16)
            for i in range(PERCORE + 1):
                # stage-1 of image i interleaved with stage-2 of image i-1.
                # First 4 s1 blocks lead so the eltwise pipeline of image
                # i-1 can drain before its first s2 pair.
                if i == PERCORE:
                    pair_seq = list(range(20)) + list(range(24, 32)) + \
                        list(range(20, 24))
                else:
                    pair_seq = list(range(NPAIR))
                prog = []
                for j in range(NBLK1):
                    if i < PERCORE:
                        prog.append(("s1", j))
                    if i >= 1:
                        prog += [("s2", pair_seq[2 * j]),
                                 ("s2", pair_seq[2 * j + 1])]
                for kind, idx in prog:
                    if kind == "s1":
                        jj = idx
                        j = POS2BLK[jj]
                        gb = NBLK1 * i + jj
                        if i == 0:
                            # img-0 quarters land in order Q3,Q0,Q1,Q2
                            QORD = {3: 1, 0: 2, 1: 3, 2: 4}
                            b = POS2BLK[jj]
                            if jj == 0 or POS2BLK[jj - 1] // 4 != b // 4:
                                tensor.wait_ge(xdma, 16 * QORD[b // 4])
                        elif jj == 0:
                            tensor.wait_ge(xdma, 48 + 16 * (i + 1))
                        if gb >= 6:
                            tensor.wait_ge(dvh, (gb - 6) // 2 + 1)
                        nc.tensor.matmul(
                            ps1[0:96, gb % 6, :], w1sb[0:32, :],
                            xs_sb[0:32, i % 2, 4 * j:4 * j + 4, :],
                            start=True, stop=True).then_inc(s1pe, 1)
                    else:
                        g = idx
                        ii = i - 1
                        gp_ = NPAIR * ii + g
                        tensor.wait_ge(dvh, NCHUNK * ii + pair_chunk(g) + 1)
                        if g == 0:
                            if ii == 0:
                                tensor.wait_ge(wdma2, 16)
                                tensor.wait_ge(mset, 6)
                            tensor.wait_ge(halo, 32 * (ii + 1))
                        if gp_ >= 16:
                            tensor.wait_ge(ocp, gp_ // 8 - 1)
                        # shadow rows for this pair's tiles must be built
                        tensor.wait_ge(
                            shd, 16 * (NCHUNK * ii + pair_chunk(g) + 1))
                        sub = g % 8
                        sl = (g // 8) % 2
                        nc.tensor.matmul(
                            ps2[0:48, sl, 0:258], w2sb[0:96, sub, 0, :, :],
                            s2mov(ii % 2, g, 0), start=(sub == 0), stop=False,
                            perf_mode=PM.DoubleRow, skip_group_check=True)
                        nc.tensor.matmul(
                            ps2[0:48, sl, 0:258], w2sb[0:96, sub, 1, :, :],
                            s2mov(ii % 2, g, 1), start=False, stop=False,
                            perf_mode=PM.DoubleRow, skip_group_check=True)
                        nc.tensor.matmul(
                            ps2[0:48, sl, 0:258], w2sb[0:96, sub, 2, :, :],
                            s2mov(ii % 2, g, 2), start=False, stop=(sub == 7),
                            perf_mode=PM.DoubleRow,
                            skip_group_check=True).then_inc(s2pe, 1)


        def emit_exp(scalar, gc):
            sp = (2 * gc) % 6
            scalar.wait_ge(s1pe, 2 * gc + 2)
            if gc >= 3:
                scalar.wait_ge(dvh, gc - 2)
            nc.scalar.activation(
                e_sb[0:96, sp:sp + 2, :],
                ps1[0:96, sp:sp + 2, :], AF.Exp,
                bias=negone[0:96, 0:1]).then_inc(acte, 1)

        def emit_ocp(scalar, i1, s, wait_pairs):
            cc = 4 * i1 + s
            scalar.wait_ge(s2pe, NPAIR * i1 + wait_pairs)
            if cc >= 4:
                scalar.wait_ge(odma, 16 * (cc // 2 - 1))
            nc.scalar.activation(
                out_sb[0:48, s, :], ps2[0:48, s % 2, 0:258],
                AF.Identity, bias=cbsb[0:48, 0:1]).then_inc(ocp, 1)

        @block.scalar
        def _(scalar):
            scalar.wait_ge(mset, 1)
            # preload the Exp PWP table off the critical path
            nc.scalar.activation(tdum[0:1, :], negone[0:1, 0:1].to_broadcast(
                (1, 2)), AF.Exp)
            scalar.wait_ge(wdma3, 16)
            for i in range(PERCORE + 1):
                for phase in range(4):
                    if i < PERCORE:
                        for k in range(2 * phase, 2 * phase + 2):
                            emit_exp(scalar, NCHUNK * i + k)
                    if i >= 1:
                        if i == PERCORE:
                            # last image: region 3 completes before region 2
                            order = [(0, 8), (1, 16), (3, 28), (2, 32)]
                            s_, wp = order[phase]
                            emit_ocp(scalar, i - 1, s_, wp)
                        else:
                            emit_ocp(scalar, i - 1, phase, 8 * (phase + 1))

        @block.vector
        def _(vector):
            for i in range(PERCORE):
                for k in range(NCHUNK):
                    gc = NCHUNK * i + k
                    sp = (2 * gc) % 6
                    vector.wait_ge(acte, gc + 1)
                    if k == 0 and i >= 2:
                        vector.wait_ge(s2pe, NPAIR * (i - 1))
                    b0 = POS2BLK[2 * k]
                    X = (4 * b0 + 1) * HCOL + 1
                    nc.vector.scalar_tensor_tensor(
                        cap(h1b[0:96, i % 2, 0, X:X + W],
                            [[PSTRIDE, 96], [HCOL, 8], [1, W]]),
                        e_sb[0:96, sp:sp + 2, :], 1.0,
                        ps1[0:96, sp:sp + 2, :],
                        ALU.min, ALU.max).then_inc(dvh, 1)

    return nc


def _prep_inputs(x, weight1, center1, bias1, weight2, center2, res):
    import ml_dtypes
    bf16 = ml_dtypes.bfloat16
    fp8 = ml_dtypes.float8_e4m3

    mask0, mask1, mask = _make_masks(C, KK)
    w1 = (weight1 * mask0 + _softplus(center1) * mask1) * mask  # [L,C,C,K,K]
    w2 = (weight2 * mask0 + _softplus(center2) * mask1) * mask
    W1 = w1.reshape(CO1, C, KK, KK).astype(np.float32)
    # V[ch=(l,ci), co, ky, kx] = w2[l, co, ci, ky, kx] / L
    V = (w2.transpose(0, 2, 1, 3, 4).reshape(CO1, C, KK, KK) / L)
    V = V.astype(np.float32)

    # stage-1 stationary [32, 96]
    w1dev = np.zeros((32, 96), np.float32)
    for t, (dy, dx) in enumerate(TAPS):
        for ci in range(C):
            w1dev[3 * t + ci, 0:CO1] = W1[:, ci, dy, dx]
            w1dev[16 + 3 * t + ci, CO1:96] = W1[:, ci, dy, dx]
    w1dev[15, 0:CO1] = bias1.reshape(CO1) + 1.0
    w1dev[31, CO1:96] = bias1.reshape(CO1) + 1.0

    # stage-2 stationaries [96, 3, 2, 6] fp8 + exact f32 correction bias
    V8 = {t: V[:, :, t[0], t[1]].astype(fp8).astype(np.float32) for t in TAPS}
    V11_lo = (V[:, :, 1, 1] - V8[(1, 1)]).astype(fp8).astype(np.float32)
    w2dev = np.zeros((96, 8, 3, 2, 48), np.float32)
    csum = np.zeros(C, np.float64)
    for p, (t0, t1) in enumerate(P_TILES):
        m0 = V8[t0]
        m1 = V11_lo if p == 2 else V8[t1]
        for sub in range(8):
            for half in range(2):
                c0 = 6 * sub + 3 * half
                w2dev[half * CO1:(half + 1) * CO1, sub, p, 0, c0:c0 + 3] = m0
                w2dev[half * CO1:(half + 1) * CO1, sub, p, 1, c0:c0 + 3] = m1
        csum += m0.sum(axis=0)
        csum += m1.sum(axis=0)
    cb = np.zeros((48, 1), np.float32)
    for sub in range(8):
        for half in range(2):
            p0 = 6 * sub + 3 * half
            cb[p0:p0 + 3, 0] = -csum
    rscale = np.float32(res[0] * (res[0] > 0))

    # pre-shifted x planes [B, 32, 64, 128]
    B = x.shape[0]
    xpad = np.zeros((B, C, H + 2, W + 2), np.float32)
    xpad[:, :, 1:H + 1, 1:W + 1] = x
    xs = np.empty((B, 32, HALF, W), np.float32)
    for t, (dy, dx) in enumerate(TAPS):
        for ci in range(C):
            xs[:, 3 * t + ci] = xpad[:, ci, dy:dy + HALF, dx:dx + W]
            xs[:, 16 + 3 * t + ci] = xpad[:, ci, HALF + dy:HALF + dy + HALF,
                                          dx:dx + W]
    xs[:, 15] = 1.0
    xs[:, 31] = 1.0
    return (xs.astype(bf16), w1dev.astype(bf16), w2dev.astype(fp8),
            cb, rscale)


def _unscramble(raw, B):
    """raw [B, 48, 4, 258] -> [B, 3, 128, 128].
    pair g: partition p = 6*(g%8) + 3*half + co, region g//8; col n = 130r+c
    (n=128,129 dead); out row = 64*half + 2*g + r."""
    out = np.empty((B, C, H, W), np.float32)
    sub = np.arange(8)
    for half in range(2):
        for co in range(C):
            p = 6 * sub + 3 * half + co              # [8]
            v = raw[:, p]                            # [B, 8, 4, 258]
            v = np.stack([v[..., 0:W], v[..., 130:130 + W]], axis=3)
            # v: [B, sub, region, r, c]; row = 64*half + 2*(8*region+sub)+r
            v = v.transpose(0, 2, 1, 3, 4)           # B, region, sub, r, c
            out[:, co, 64 * half:64 * half + 64] = v.reshape(B, 64, W)
    return out


def kernel(x, weight1, center1, bias1, weight2, center2, res, _trace=False):
    from concourse.bass_utils import run_bass_kernel_spmd

    x = np.asarray(x, np.float32)
    xs, w1dev, w2dev, cb, rscale = _prep_inputs(
        x, np.asarray(weight1, np.float32),
        np.asarray(center1, np.float32), np.asarray(bias1, np.float32),
        np.asarray(weight2, np.float32), np.asarray(center2, np.float32),
        np.asarray(res, np.float32))

    if "nc" not in _CACHE:
        _CACHE["nc"] = _build_nc()
    nc = _CACHE["nc"]

    in_maps = [
        {"xs": xs[i * PERCORE:(i + 1) * PERCORE], "w1": w1dev, "w2": w2dev,
         "cb": cb}
        for i in range(N_CORES)
    ]
    res_ = run_bass_kernel_spmd(nc, in_maps, list(range(N_CORES)),
                                trace=_trace)
    raw = np.concatenate([r["out"] for r in res_.results], axis=0)
    out = _unscramble(raw, x.shape[0]) + rscale * x
    if _trace:
        _CACHE["exec_time_ns"] = res_.exec_time_ns
        _CACHE["profile"] = res_.profile_json
    return out
